# revision 19
# baseline (speedup 1.0000x reference)
"""Chamfer loss kernel for Trainium2 (8 NeuronCores, SPMD).

Problem: template (4, 8192, 3) fp32, source (4, 8192, 3) fp32.
loss = mean_n min_m ||t_n - s_m|| + mean_m min_n ||t_n - s_m||  (means over
B*N = 32768 values each).

Strategy (v2)
-------------
Each core owns one (batch, half) pair and both directions (64 row tiles of
128 query points).  A K=13 matmul with every operand split into fp16 hi+lo
pairs emits d^2 tiles directly into PSUM at fp32-level precision (~1e-5):
   k 0-2 : -2*q_hi  |  r_hi        k 9  : q2_lo | 1
   k 3-5 : -2*q_hi  |  r_lo        k 10 : 1     | r2_hi
   k 6-8 : -2*q_lo  |  r_hi        k 11 : 1     | r2_lo   (k 12: q2_hi | 1)

PE: row tiles run 4 at a time via tile_position row-group packing (K=13
fits one 32-row strip); the 4 concurrent matmuls write 4 different PSUM
banks, so a sweep-chunk costs about one matmul.

The PSUM drain is the true bottleneck (every d^2 entry must cross one of
the two PSUM read ports), so it is split across both port owners:
 - DIRECT chunks: DVE tensor_reduce(min) straight from PSUM (1 elem/cyc/ln)
 - OFFLOAD chunks: ScalarE copies PSUM -> SBUF fp16 (1 elem/cyc/ln @1.2GHz,
   free downcast; d^2 near 0 keeps full relative precision in fp16), then
   DVE folds the fp16 staging with pairwise tensor_tensor(min) at 2
   results/cyc (2x_1P mode) -- ~0.55 cyc/entry.
Mins are clamped at 1e-12, sqrt'd on ScalarE, summed per partition; the
host sums the 8 [128,1] partials and divides by 32768.
"""

import numpy as np

import concourse.bass as bass
import concourse.mybir as mybir
from concourse.bass_utils import run_bass_kernel_spmd
from concourse.tile import TileContext

B, N = 4, 8192
HALF = N // 2            # query rows per core per direction
K = 13                   # split-fp16 contraction depth
CHUNK = 512              # one PSUM bank
N_CHUNKS = N // CHUNK    # 16
DIRECT = 4               # chunks 0..DIRECT-1 drain via DVE; rest via ScalarE
OFF = N_CHUNKS - DIRECT  # offloaded chunks (12)
GROUPS = 4               # row-group packing factor
SWEEPS = 2 * (HALF // 128) // GROUPS   # 16 sweeps of 4 row tiles
N_CORES = 8

# inp column layout (fp16, [128, IN_COLS])
LW = HALF // GROUPS                      # lhsT cols per direction = 1024
C_L0, C_L1 = 0, LW                        # lhsT dir0 / dir1
C_R0, C_R1 = 2 * LW, 2 * LW + N           # rhs dir0 / dir1
IN_COLS = 2 * LW + 2 * N


def _split16(x):
    hi = x.astype(np.float16)
    lo = (x.astype(np.float32) - hi.astype(np.float32)).astype(np.float16)
    return hi, lo


def _build_lhsT(q):
    """q: (n, 3) fp32 -> (13, n) fp16 stationary operand."""
    n = q.shape[0]
    qh, ql = _split16(q)
    qhat = qh.astype(np.float32) + ql.astype(np.float32)
    q2 = (qhat ** 2).sum(1)
    q2h, q2l = _split16(q2)
    out = np.empty((K, n), np.float16)
    out[0:3] = (-2.0 * qh.astype(np.float32)).astype(np.float16).T
    out[3:6] = out[0:3]
    out[6:9] = (-2.0 * ql.astype(np.float32)).astype(np.float16).T
    out[9] = q2h
    out[10] = q2l
    out[11] = 1.0
    out[12] = 1.0
    return out


def _build_rhs(r):
    """r: (m, 3) fp32 -> (13, m) fp16 moving operand."""
    m = r.shape[0]
    rh, rl = _split16(r)
    rhat = rh.astype(np.float32) + rl.astype(np.float32)
    r2 = (rhat ** 2).sum(1)
    r2h, r2l = _split16(r2)
    out = np.empty((K, m), np.float16)
    out[0:3] = rh.T
    out[3:6] = rl.T
    out[6:9] = rh.T
    out[9] = 1.0
    out[10] = 1.0
    out[11] = r2h
    out[12] = r2l
    return out


def _build_program():
    nc = bass.Bass()
    f16, f32 = mybir.dt.float16, mybir.dt.float32

    inp_d = nc.dram_tensor("inp", [128, IN_COLS], f16, kind="ExternalInput")
    out_d = nc.dram_tensor("out", [128, 1], f32, kind="ExternalOutput")

    with TileContext(nc) as tc:
        with (
            tc.tile_pool(name="singles", bufs=1) as singles,
            tc.tile_pool(name="psum", bufs=2, space="PSUM") as psum_pool,
        ):
            inp_sb = singles.tile([128, IN_COLS], f16)
            # direction 0 data first (lhsT, then rhs in 4 parallel queue
            # chunks) so the first sweeps start as early as possible
            nc.sync.dma_start(out=inp_sb[:, C_L0:C_L0 + LW],
                              in_=inp_d[:, C_L0:C_L0 + LW])
            for c in range(4):
                c0 = C_R0 + c * (N // 4)
                nc.sync.dma_start(out=inp_sb[:, c0:c0 + N // 4],
                                  in_=inp_d[:, c0:c0 + N // 4])
            nc.sync.dma_start(out=inp_sb[:, C_L1:C_L1 + LW],
                              in_=inp_d[:, C_L1:C_L1 + LW])
            nc.sync.dma_start(out=inp_sb[:, C_R1:C_R1 + N],
                              in_=inp_d[:, C_R1:C_R1 + N])

            mind = singles.tile([128, SWEEPS, GROUPS, DIRECT], f32)
            mint = singles.tile([128, SWEEPS, GROUPS], f16)

            # chunk j -> consumer: every 4th chunk drains directly on DVE so
            # both PSUM readers (DVE, ScalarE) stay busy concurrently.  The
            # sweep starts and ends with ScalarE chunks so the previous
            # sweep's DVE fold-tail overlaps fresh copies instead of
            # holding the PSUM slot pipeline.
            is_direct = [j % (N_CHUNKS // DIRECT) == 1 for j in range(N_CHUNKS)]

            for s in range(SWEEPS):
                direction, sl = divmod(s, SWEEPS // 2)
                lbase = (C_L0 if direction == 0 else C_L1) + sl * 128
                rbase = C_R0 if direction == 0 else C_R1
                pairs = singles.tile([128, OFF // 2, GROUPS, CHUNK], f16,
                                     tag="pairs", bufs=2)
                ndir = 0
                noff = 0
                pending = None
                for j in range(N_CHUNKS):
                    ps = psum_pool.tile([128, GROUPS * CHUNK], f32, tag="ps")
                    for g in range(GROUPS):
                        nc.tensor.matmul(
                            ps[:, g * CHUNK:(g + 1) * CHUNK],
                            inp_sb[32 * g:32 * g + K, lbase:lbase + 128],
                            inp_sb[32 * g:32 * g + K,
                                   rbase + j * CHUNK:rbase + (j + 1) * CHUNK],
                            start=True, stop=True,
                            tile_position=(32 * g, 0),
                        )
                    if is_direct[j]:
                        # two half reduces -> banks 0-1 release earlier, so
                        # the next generation's matmuls start sooner
                        nc.vector.tensor_reduce(
                            out=mind[:, s, 0:2, ndir],
                            in_=ps[:, 0:2 * CHUNK].rearrange(
                                "p (g x) -> p g x", x=CHUNK),
                            axis=mybir.AxisListType.X,
                            op=mybir.AluOpType.min,
                        )
                        nc.vector.tensor_reduce(
                            out=mind[:, s, 2:4, ndir],
                            in_=ps[:, 2 * CHUNK:4 * CHUNK].rearrange(
                                "p (g x) -> p g x", x=CHUNK),
                            axis=mybir.AxisListType.X,
                            op=mybir.AluOpType.min,
                        )
                        ndir += 1
                    else:
                        st = singles.tile([128, GROUPS, CHUNK], f16,
                                          tag="stag", bufs=6)
                        nc.scalar.copy(out=st, in_=ps)
                        if pending is None:
                            pending = st
                        else:
                            # fold two staged chunks as soon as both exist
                            nc.vector.tensor_tensor(
                                out=pairs[:, noff, :, :], in0=pending, in1=st,
                                op=mybir.AluOpType.min)
                            noff += 1
                            pending = None

                # sweep tail: fold the 6 pair tiles -> [128, GROUPS] fp16
                t3 = singles.tile([128, OFF // 4, GROUPS, CHUNK], f16,
                                  tag="t3")
                nc.vector.tensor_tensor(
                    out=t3, in0=pairs[:, 0:OFF // 4, :, :],
                    in1=pairs[:, OFF // 4:OFF // 2, :, :],
                    op=mybir.AluOpType.min)
                t1 = singles.tile([128, GROUPS, CHUNK], f16, tag="t1")
                nc.vector.tensor_tensor(
                    out=t1, in0=t3[:, 0, :, :], in1=t3[:, 1, :, :],
                    op=mybir.AluOpType.min)
                nc.vector.tensor_tensor(
                    out=t1, in0=t1, in1=t3[:, 2, :, :],
                    op=mybir.AluOpType.min)
                nc.vector.tensor_reduce(
                    out=mint[:, s, :], in_=t1,
                    axis=mybir.AxisListType.X, op=mybir.AluOpType.min)

            # tail: fold direct mins, combine with tree mins, sqrt, sum
            md = singles.tile([128, SWEEPS, GROUPS], f32)
            nc.vector.tensor_reduce(
                out=md, in_=mind, axis=mybir.AxisListType.X,
                op=mybir.AluOpType.min)
            mfin = singles.tile([128, SWEEPS * GROUPS], f32)
            nc.vector.tensor_tensor(
                out=mfin.rearrange("p (s g) -> p s g", g=GROUPS),
                in0=md, in1=mint, op=mybir.AluOpType.min)
            nc.vector.tensor_scalar_max(out=mfin, in0=mfin, scalar1=1e-12)
            dfin = singles.tile([128, SWEEPS * GROUPS], f32)
            nc.scalar.activation(
                out=dfin, in_=mfin, func=mybir.ActivationFunctionType.Sqrt)
            acc = singles.tile([128, 1], f32)
            nc.vector.tensor_reduce(
                out=acc, in_=dfin, axis=mybir.AxisListType.X,
                op=mybir.AluOpType.add)
            nc.sync.dma_start(out=out_d[:, :], in_=acc)

    _strip_redundant_pe_waits(nc)
    return nc


def _strip_redundant_pe_waits(nc):
    """Several ISA instruction structs hold only ONE sync-wait command, but
    Tile's sem assignment is per-proc minimal, NOT transitively minimal:
    e.g. the first matmul of a recycled PSUM slot gets {PE>=a (bank WAR),
    DVE>=b (the reduce that drained the slot)} even though observing DVE>=b
    already implies PE>=a; the kernel-tail Drain waits on every proc.
    walrus then fails codegen with "Too many sync wait commands".

    Pass 1 is a sound transitive reduction: a wait S>=v may be dropped when
    the closure of the remaining waits implies it (observing S2>=v2 means
    the instruction whose cumulative inc of S2 reached v2 completed, hence
    its own waits -- and those of every earlier instruction on its issuing
    engine, which dispatches in order -- were satisfied).
    Pass 2 moves any still-excess waits onto injected same-engine NoOps."""
    order = []
    for f in nc.m.functions:
        for blk in f.blocks:
            order.extend(blk.instructions)

    engine_run = {}
    tick = {}
    prefmax = {}
    for ins in order:
        si = ins.sync_info
        eng = str(ins.engine)
        run = engine_run.setdefault(eng, {})
        if si is not None and si.on_wait:
            for w in si.on_wait:
                if run.get(w.ant_name, 0) < w.wait_value:
                    run[w.ant_name] = w.wait_value
        if si is not None and si.on_update:
            for u in si.on_update:
                if u.update_mode in ("sem-inc", "sem-add-imm") and u.update_value:
                    t = tick.get(u.ant_name, 0) + u.update_value
                    tick[u.ant_name] = t
                    prefmax.setdefault(u.ant_name, []).append((t, dict(run)))

    def closure(start):
        known = {}
        for (s, v) in start:
            if known.get(s, 0) < v:
                known[s] = v
        for _ in range(16):
            changed = False
            for s, v in list(known.items()):
                lst = prefmax.get(s)
                if not lst:
                    continue
                lo, hi = 0, len(lst)
                while lo < hi:
                    mid = (lo + hi) // 2
                    if lst[mid][0] <= v:
                        lo = mid + 1
                    else:
                        hi = mid
                if lo == 0:
                    continue
                for s2, v2 in lst[lo - 1][1].items():
                    if known.get(s2, 0) < v2:
                        known[s2] = v2
                        changed = True
            if not changed:
                break
        return known

    for ins in order:
        si = ins.sync_info
        if si is None or not si.on_wait or len(si.on_wait) <= 1:
            continue
        kept = list(si.on_wait)
        changed = True
        while changed and len(kept) > 1:
            changed = False
            for w in list(kept):
                known = closure([(x.ant_name, x.wait_value)
                                 for x in kept if x is not w])
                if known.get(w.ant_name, 0) >= w.wait_value:
                    kept.remove(w)
                    changed = True
                    break
        if len(kept) < len(si.on_wait):
            si.on_wait = kept
            ins.sync_info = si

    # Fallback: move excess waits onto injected same-engine NoOps (engines
    # dispatch in order, so a wait satisfied on the NoOp also gates the
    # following instruction).
    nop_id = [0]
    for f in nc.m.functions:
        for blk in f.blocks:
            lst = blk.instructions
            i = 0
            while i < len(lst):
                ins = lst[i]
                si = ins.sync_info
                if si is not None and si.on_wait and len(si.on_wait) > 1:
                    extra = list(si.on_wait[:-1])
                    si.on_wait = [si.on_wait[-1]]
                    ins.sync_info = si
                    for w in extra:
                        nop_id[0] += 1
                        nop = mybir.InstNoOp(
                            name=f"waitnop-{nop_id[0]}", ins=[], outs=[])
                        nop.engine = ins.engine
                        nop.sync_info = mybir.SyncInfo(
                            on_wait=[w], on_update=[])
                        lst.insert(i, nop)
                        i += 1
                i += 1
    return nc


_PROGRAM = None


def _get_program():
    global _PROGRAM
    if _PROGRAM is None:
        _PROGRAM = _build_program()
    return _PROGRAM


def _make_in_maps(template, source):
    template = np.asarray(template, dtype=np.float32)
    source = np.asarray(source, dtype=np.float32)
    in_maps = []
    for core in range(N_CORES):
        b, h = divmod(core, 2)
        r0, r1 = h * HALF, (h + 1) * HALF
        inp = np.zeros((128, IN_COLS), np.float16)
        for direction, (q, r) in enumerate(
            [(template[b, r0:r1], source[b]),
             (source[b, r0:r1], template[b])]
        ):
            lhsT = _build_lhsT(q)        # (13, 4096)
            rhs = _build_rhs(r)          # (13, 8192)
            cl = C_L0 if direction == 0 else C_L1
            cr = C_R0 if direction == 0 else C_R1
            # row tile (sl*4+g) lives at partitions 32g+0..12,
            # cols cl + sl*128 ...
            lhsT_t = lhsT.reshape(K, SWEEPS // 2, GROUPS, 128)
            for g in range(GROUPS):
                inp[32 * g:32 * g + K, cl:cl + LW] = (
                    lhsT_t[:, :, g, :].reshape(K, LW))
                inp[32 * g:32 * g + K, cr:cr + N] = rhs
        in_maps.append({"inp": inp})
    return in_maps


def _run(template, source, trace=False):
    nc = _get_program()
    in_maps = _make_in_maps(template, source)
    res = run_bass_kernel_spmd(nc, in_maps, list(range(N_CORES)), trace=trace)
    total = np.float64(0.0)
    for r in res.results:
        total += np.asarray(r["out"], dtype=np.float64).sum()
    loss = np.float32(total / (B * N))
    return loss, res


def kernel(template, source):
    loss, _ = _run(template, source, trace=False)
    return loss


# revision 20
# speedup vs baseline: 1.0375x; 1.0375x over previous
"""Chamfer loss kernel for Trainium2 (8 NeuronCores, SPMD).

Problem: template (4, 8192, 3) fp32, source (4, 8192, 3) fp32.
loss = mean_n min_m ||t_n - s_m|| + mean_m min_n ||t_n - s_m||  (means over
B*N = 32768 values each).

Strategy (v2)
-------------
Each core owns one (batch, half) pair and both directions (64 row tiles of
128 query points).  A K=13 matmul with every operand split into fp16 hi+lo
pairs emits d^2 tiles directly into PSUM at fp32-level precision (~1e-5):
   k 0-2 : -2*q_hi  |  r_hi        k 9  : q2_lo | 1
   k 3-5 : -2*q_hi  |  r_lo        k 10 : 1     | r2_hi
   k 6-8 : -2*q_lo  |  r_hi        k 11 : 1     | r2_lo   (k 12: q2_hi | 1)

PE: row tiles run 4 at a time via tile_position row-group packing (K=13
fits one 32-row strip); the 4 concurrent matmuls write 4 different PSUM
banks, so a sweep-chunk costs about one matmul.

The PSUM drain is the true bottleneck (every d^2 entry must cross one of
the two PSUM read ports), so it is split across both port owners:
 - DIRECT chunks: DVE tensor_reduce(min) straight from PSUM (1 elem/cyc/ln)
 - OFFLOAD chunks: ScalarE copies PSUM -> SBUF fp16 (1 elem/cyc/ln @1.2GHz,
   free downcast; d^2 near 0 keeps full relative precision in fp16), then
   DVE folds the fp16 staging with pairwise tensor_tensor(min) at 2
   results/cyc (2x_1P mode) -- ~0.55 cyc/entry.
Mins are clamped at 1e-12, sqrt'd on ScalarE, summed per partition; the
host sums the 8 [128,1] partials and divides by 32768.
"""

import numpy as np

import concourse.bass as bass
import concourse.mybir as mybir
from concourse.bass_utils import run_bass_kernel_spmd
from concourse.tile import TileContext

B, N = 4, 8192
HALF = N // 2            # query rows per core per direction
K = 13                   # split-fp16 contraction depth
CHUNK = 512              # one PSUM bank
N_CHUNKS = N // CHUNK    # 16
DIRECT = 4               # chunks 0..DIRECT-1 drain via DVE; rest via ScalarE
OFF = N_CHUNKS - DIRECT  # offloaded chunks (12)
GROUPS = 4               # row-group packing factor
SWEEPS = 2 * (HALF // 128) // GROUPS   # 16 sweeps of 4 row tiles
N_CORES = 8

# inp column layout (fp16, [128, IN_COLS])
LW = HALF // GROUPS                      # lhsT cols per direction = 1024
C_L0, C_L1 = 0, LW                        # lhsT dir0 / dir1
C_R0, C_R1 = 2 * LW, 2 * LW + N           # rhs dir0 / dir1
IN_COLS = 2 * LW + 2 * N


def _split16(x):
    hi = x.astype(np.float16)
    lo = (x.astype(np.float32) - hi.astype(np.float32)).astype(np.float16)
    return hi, lo


def _build_lhsT(q):
    """q: (n, 3) fp32 -> (13, n) fp16 stationary operand."""
    n = q.shape[0]
    qh, ql = _split16(q)
    qhat = qh.astype(np.float32) + ql.astype(np.float32)
    q2 = (qhat ** 2).sum(1)
    q2h, q2l = _split16(q2)
    out = np.empty((K, n), np.float16)
    out[0:3] = (-2.0 * qh.astype(np.float32)).astype(np.float16).T
    out[3:6] = out[0:3]
    out[6:9] = (-2.0 * ql.astype(np.float32)).astype(np.float16).T
    out[9] = q2h
    out[10] = q2l
    out[11] = 1.0
    out[12] = 1.0
    return out


def _build_rhs(r):
    """r: (m, 3) fp32 -> (13, m) fp16 moving operand."""
    m = r.shape[0]
    rh, rl = _split16(r)
    rhat = rh.astype(np.float32) + rl.astype(np.float32)
    r2 = (rhat ** 2).sum(1)
    r2h, r2l = _split16(r2)
    out = np.empty((K, m), np.float16)
    out[0:3] = rh.T
    out[3:6] = rl.T
    out[6:9] = rh.T
    out[9] = 1.0
    out[10] = 1.0
    out[11] = r2h
    out[12] = r2l
    return out


def _build_program():
    nc = bass.Bass()
    f16, f32 = mybir.dt.float16, mybir.dt.float32

    inp_d = nc.dram_tensor("inp", [128, IN_COLS], f16, kind="ExternalInput")
    out_d = nc.dram_tensor("out", [128, 1], f32, kind="ExternalOutput")

    with TileContext(nc) as tc:
        with (
            tc.tile_pool(name="singles", bufs=1) as singles,
            tc.tile_pool(name="psum", bufs=2, space="PSUM") as psum_pool,
        ):
            inp_sb = singles.tile([128, IN_COLS], f16)
            # direction 0 data first (lhsT, then rhs in 4 parallel queue
            # chunks) so the first sweeps start as early as possible
            nc.sync.dma_start(out=inp_sb[:, C_L0:C_L0 + LW],
                              in_=inp_d[:, C_L0:C_L0 + LW])
            for c in range(4):
                c0 = C_R0 + c * (N // 4)
                nc.sync.dma_start(out=inp_sb[:, c0:c0 + N // 4],
                                  in_=inp_d[:, c0:c0 + N // 4])
            nc.sync.dma_start(out=inp_sb[:, C_L1:C_L1 + LW],
                              in_=inp_d[:, C_L1:C_L1 + LW])
            nc.sync.dma_start(out=inp_sb[:, C_R1:C_R1 + N],
                              in_=inp_d[:, C_R1:C_R1 + N])

            mind = singles.tile([128, SWEEPS, GROUPS, DIRECT], f32)
            mint = singles.tile([128, SWEEPS, GROUPS], f16)

            # chunk j -> consumer: every 4th chunk drains directly on DVE so
            # both PSUM readers (DVE, ScalarE) stay busy concurrently.  The
            # sweep starts and ends with ScalarE chunks so the previous
            # sweep's DVE fold-tail overlaps fresh copies instead of
            # holding the PSUM slot pipeline.
            is_direct = [j % (N_CHUNKS // DIRECT) == 1 for j in range(N_CHUNKS)]

            for s in range(SWEEPS):
                direction, sl = divmod(s, SWEEPS // 2)
                lbase = (C_L0 if direction == 0 else C_L1) + sl * 128
                rbase = C_R0 if direction == 0 else C_R1
                pairs = singles.tile([128, OFF // 2, GROUPS, CHUNK], f16,
                                     tag="pairs", bufs=2)
                ndir = 0
                noff = 0
                pending = None
                for j in range(N_CHUNKS):
                    ps = psum_pool.tile([128, GROUPS * CHUNK], f32, tag="ps")
                    for g in range(GROUPS):
                        nc.tensor.matmul(
                            ps[:, g * CHUNK:(g + 1) * CHUNK],
                            inp_sb[32 * g:32 * g + K, lbase:lbase + 128],
                            inp_sb[32 * g:32 * g + K,
                                   rbase + j * CHUNK:rbase + (j + 1) * CHUNK],
                            start=True, stop=True,
                            tile_position=(32 * g, 0),
                        )
                    if is_direct[j]:
                        nc.vector.tensor_reduce(
                            out=mind[:, s, :, ndir],
                            in_=ps.rearrange("p (g x) -> p g x", x=CHUNK),
                            axis=mybir.AxisListType.X,
                            op=mybir.AluOpType.min,
                        )
                        ndir += 1
                    else:
                        st = singles.tile([128, GROUPS, CHUNK], f16,
                                          tag="stag", bufs=6)
                        nc.scalar.copy(out=st, in_=ps)
                        if pending is None:
                            pending = st
                        else:
                            # fold two staged chunks as soon as both exist
                            nc.vector.tensor_tensor(
                                out=pairs[:, noff, :, :], in0=pending, in1=st,
                                op=mybir.AluOpType.min)
                            noff += 1
                            pending = None

                # sweep tail: fold the 6 pair tiles -> [128, GROUPS] fp16
                t3 = singles.tile([128, OFF // 4, GROUPS, CHUNK], f16,
                                  tag="t3")
                nc.vector.tensor_tensor(
                    out=t3, in0=pairs[:, 0:OFF // 4, :, :],
                    in1=pairs[:, OFF // 4:OFF // 2, :, :],
                    op=mybir.AluOpType.min)
                t1 = singles.tile([128, GROUPS, CHUNK], f16, tag="t1")
                nc.vector.tensor_tensor(
                    out=t1, in0=t3[:, 0, :, :], in1=t3[:, 1, :, :],
                    op=mybir.AluOpType.min)
                nc.vector.tensor_tensor(
                    out=t1, in0=t1, in1=t3[:, 2, :, :],
                    op=mybir.AluOpType.min)
                nc.vector.tensor_reduce(
                    out=mint[:, s, :], in_=t1,
                    axis=mybir.AxisListType.X, op=mybir.AluOpType.min)

            # tail: fold direct mins, combine with tree mins, sqrt, sum
            md = singles.tile([128, SWEEPS, GROUPS], f32)
            nc.vector.tensor_reduce(
                out=md, in_=mind, axis=mybir.AxisListType.X,
                op=mybir.AluOpType.min)
            mfin = singles.tile([128, SWEEPS * GROUPS], f32)
            nc.vector.tensor_tensor(
                out=mfin.rearrange("p (s g) -> p s g", g=GROUPS),
                in0=md, in1=mint, op=mybir.AluOpType.min)
            nc.vector.tensor_scalar_max(out=mfin, in0=mfin, scalar1=1e-12)
            dfin = singles.tile([128, SWEEPS * GROUPS], f32)
            nc.scalar.activation(
                out=dfin, in_=mfin, func=mybir.ActivationFunctionType.Sqrt)
            acc = singles.tile([128, 1], f32)
            nc.vector.tensor_reduce(
                out=acc, in_=dfin, axis=mybir.AxisListType.X,
                op=mybir.AluOpType.add)
            nc.sync.dma_start(out=out_d[:, :], in_=acc)

    _strip_redundant_pe_waits(nc)
    return nc


def _strip_redundant_pe_waits(nc):
    """Several ISA instruction structs hold only ONE sync-wait command, but
    Tile's sem assignment is per-proc minimal, NOT transitively minimal:
    e.g. the first matmul of a recycled PSUM slot gets {PE>=a (bank WAR),
    DVE>=b (the reduce that drained the slot)} even though observing DVE>=b
    already implies PE>=a; the kernel-tail Drain waits on every proc.
    walrus then fails codegen with "Too many sync wait commands".

    Pass 1 is a sound transitive reduction: a wait S>=v may be dropped when
    the closure of the remaining waits implies it (observing S2>=v2 means
    the instruction whose cumulative inc of S2 reached v2 completed, hence
    its own waits -- and those of every earlier instruction on its issuing
    engine, which dispatches in order -- were satisfied).
    Pass 2 moves any still-excess waits onto injected same-engine NoOps."""
    order = []
    for f in nc.m.functions:
        for blk in f.blocks:
            order.extend(blk.instructions)

    engine_run = {}
    tick = {}
    prefmax = {}
    for ins in order:
        si = ins.sync_info
        eng = str(ins.engine)
        run = engine_run.setdefault(eng, {})
        if si is not None and si.on_wait:
            for w in si.on_wait:
                if run.get(w.ant_name, 0) < w.wait_value:
                    run[w.ant_name] = w.wait_value
        if si is not None and si.on_update:
            for u in si.on_update:
                if u.update_mode in ("sem-inc", "sem-add-imm") and u.update_value:
                    t = tick.get(u.ant_name, 0) + u.update_value
                    tick[u.ant_name] = t
                    prefmax.setdefault(u.ant_name, []).append((t, dict(run)))

    def closure(start):
        known = {}
        for (s, v) in start:
            if known.get(s, 0) < v:
                known[s] = v
        for _ in range(16):
            changed = False
            for s, v in list(known.items()):
                lst = prefmax.get(s)
                if not lst:
                    continue
                lo, hi = 0, len(lst)
                while lo < hi:
                    mid = (lo + hi) // 2
                    if lst[mid][0] <= v:
                        lo = mid + 1
                    else:
                        hi = mid
                if lo == 0:
                    continue
                for s2, v2 in lst[lo - 1][1].items():
                    if known.get(s2, 0) < v2:
                        known[s2] = v2
                        changed = True
            if not changed:
                break
        return known

    for ins in order:
        si = ins.sync_info
        if si is None or not si.on_wait or len(si.on_wait) <= 1:
            continue
        kept = list(si.on_wait)
        changed = True
        while changed and len(kept) > 1:
            changed = False
            for w in list(kept):
                known = closure([(x.ant_name, x.wait_value)
                                 for x in kept if x is not w])
                if known.get(w.ant_name, 0) >= w.wait_value:
                    kept.remove(w)
                    changed = True
                    break
        if len(kept) < len(si.on_wait):
            si.on_wait = kept
            ins.sync_info = si

    # Fallback: move excess waits onto injected same-engine NoOps (engines
    # dispatch in order, so a wait satisfied on the NoOp also gates the
    # following instruction).
    nop_id = [0]
    for f in nc.m.functions:
        for blk in f.blocks:
            lst = blk.instructions
            i = 0
            while i < len(lst):
                ins = lst[i]
                si = ins.sync_info
                if si is not None and si.on_wait and len(si.on_wait) > 1:
                    extra = list(si.on_wait[:-1])
                    si.on_wait = [si.on_wait[-1]]
                    ins.sync_info = si
                    for w in extra:
                        nop_id[0] += 1
                        nop = mybir.InstNoOp(
                            name=f"waitnop-{nop_id[0]}", ins=[], outs=[])
                        nop.engine = ins.engine
                        nop.sync_info = mybir.SyncInfo(
                            on_wait=[w], on_update=[])
                        lst.insert(i, nop)
                        i += 1
                i += 1
    return nc


_PROGRAM = None


def _get_program():
    global _PROGRAM
    if _PROGRAM is None:
        _PROGRAM = _build_program()
    return _PROGRAM


def _make_in_maps(template, source):
    template = np.asarray(template, dtype=np.float32)
    source = np.asarray(source, dtype=np.float32)
    in_maps = []
    for core in range(N_CORES):
        b, h = divmod(core, 2)
        r0, r1 = h * HALF, (h + 1) * HALF
        inp = np.zeros((128, IN_COLS), np.float16)
        for direction, (q, r) in enumerate(
            [(template[b, r0:r1], source[b]),
             (source[b, r0:r1], template[b])]
        ):
            lhsT = _build_lhsT(q)        # (13, 4096)
            rhs = _build_rhs(r)          # (13, 8192)
            cl = C_L0 if direction == 0 else C_L1
            cr = C_R0 if direction == 0 else C_R1
            # row tile (sl*4+g) lives at partitions 32g+0..12,
            # cols cl + sl*128 ...
            lhsT_t = lhsT.reshape(K, SWEEPS // 2, GROUPS, 128)
            for g in range(GROUPS):
                inp[32 * g:32 * g + K, cl:cl + LW] = (
                    lhsT_t[:, :, g, :].reshape(K, LW))
                inp[32 * g:32 * g + K, cr:cr + N] = rhs
        in_maps.append({"inp": inp})
    return in_maps


def _run(template, source, trace=False):
    nc = _get_program()
    in_maps = _make_in_maps(template, source)
    res = run_bass_kernel_spmd(nc, in_maps, list(range(N_CORES)), trace=trace)
    total = np.float64(0.0)
    for r in res.results:
        total += np.asarray(r["out"], dtype=np.float64).sum()
    loss = np.float32(total / (B * N))
    return loss, res


def kernel(template, source):
    loss, _ = _run(template, source, trace=False)
    return loss
